# revision 12
# baseline (speedup 1.0000x reference)
"""LocalWindowAttention Trainium2 kernel (v2).

Problem: B=8, S=4096, D=1024, H=16 heads, hd=64, window W=64.
  qkv = x @ qkv_w + qkv_b; per-window attention with relative position
  bias; out = attn_out @ proj_w + proj_b.

Sharding: data-parallel over batch — one batch element per NeuronCore
(8 cores), no collectives needed.

v2 changes vs v1 (all aimed at keeping the PE instruction stream dense so
the tensor engine stays at its top p-state, and at cutting DVE/ACT work):
  1. x is uploaded pre-transposed (feature-major) from the host — the 256
     PE transposes + 256 DVE copies of stage 1 are gone.
  2. rel-pos bias is applied multiplicatively AFTER exp:
     exp(s+b) = exp(s)*exp(b). ACT reads score PSUM tiles directly
     (no DVE fp32 bias-add, no scb SBUF round-trip); the exp(b) table
     multiply is one fp16 all-SBUF DVE op per head-group (4x DVE mode).
     Masked cross-window quadrants have exp(-1e4)=0 exactly.
  3. softmax normalize is a single DVE tensor_scalar divide per head
     (out = unnorm/denom), replacing reciprocal+tensor_scalar_mul.
  4. attn-out transpose for the proj matmul runs on the DMA XBAR
     (SBUF->SBUF dma_start_transpose) instead of PE+DVE.
  5. qkv weights are DMA'd in first-use order (q column-chunks first) so
     the first q matmul starts ~1 us in instead of waiting for 6 MB.
  6. PSUM banks freed by (1)/(4) deepen the score/attn-v pools (3 deep).

Per-core pipeline (S=4096 rows, processed in s-tiles of 512 rows):
  qT/kT (feature-major) and v (seq-major) via fp16 matmuls vs resident
  qkv_w tiles; fp32 PSUM accumulation over the K=1024 contraction.
  Attention per 128-row block (= 2 windows of 64, independent via the
  zeroed quadrants of the exp-bias table) and per group of 4 heads:
      scoresT[k,q] = kT.T @ qT          (PE, PSUM)
      att0 = exp(scoresT)               (ACT, PSUM->SBUF fp16)
      att = att0 * exp_bias             (DVE fp16, 4x mode)
      outT_unnorm[q,hd], denom[q] = attT.T @ [v | 1]   (ones column fused)
      attn_out[q,hd] = outT_unnorm / denom             (DVE divide)
  attn_out DMA-transposed per 128-col block; proj matmul; DMA out.

Scale 1/sqrt(hd) is folded into qkv_w's q-columns host-side. qkv_b's
v-part is folded into an effective proj bias host-side (rows of attn sum
to 1). All matmul operands are fp16 (error ~8e-4 vs fp32 reference);
accumulation is always fp32.
"""
import numpy as np

import concourse.bacc as bacc
import concourse.mybir as mybir
from concourse.tile import TileContext
from concourse.bass_utils import run_bass_kernel_spmd
from concourse.masks import make_identity

import os
# PE operand dtype: bf16 by default (smaller mantissa multiplier draws less
# PE power than fp16 -> less power-throttling at the chip cap). Error is
# ~6e-3 vs the 2e-2 gate. Set BASS_LWA_FP16=1 to switch back to fp16.
if os.environ.get("BASS_LWA_FP16"):
    F16 = mybir.dt.float16
    NPDT = np.float16
else:
    F16 = mybir.dt.bfloat16
    NPDT = None  # ml_dtypes.bfloat16, resolved lazily below
    import ml_dtypes
    NPDT = ml_dtypes.bfloat16
F32 = mybir.dt.float32

B, S, D = 8, 4096, 1024
H, W, HD = 16, 64, 64
NW = S // W              # 64 windows
STILE = 512              # seq rows per pipeline tile
NST = S // STILE         # 8 s-tiles
NBLK = STILE // 128      # 4 row-blocks (window pairs) per s-tile
MASK = -10000.0          # exp() underflows to exactly 0 in fp16/fp32


def _build(n_stiles=NST, with_qkbias=False, with_projbias=False):
    nc = bacc.Bacc()
    s_total = n_stiles * STILE

    xT_ext = nc.declare_dram_parameter("xt16", [D, s_total], F16, isOutput=False)
    w_ext = nc.declare_dram_parameter("qkvw16", [D, 3 * D], F16, isOutput=False)
    pw_ext = nc.declare_dram_parameter("projw16", [D, D], F16, isOutput=False)
    e2_ext = nc.declare_dram_parameter("expb2t16", [128, H * 128], F16,
                                       isOutput=False)
    out_ext = nc.declare_dram_parameter("out", [s_total, D], F32, isOutput=True)
    if with_qkbias:
        qkb_ext = nc.declare_dram_parameter("qkb", [16, 128, 1], F32,
                                            isOutput=False)
    if with_projbias:
        pbb_ext = nc.declare_dram_parameter("projb_bcast", [128, D], F32,
                                            isOutput=False)

    with TileContext(nc) as tc:
        with (
            tc.tile_pool(name="const", bufs=1) as const,
            tc.tile_pool(name="xtp", bufs=3) as xtp,
            tc.tile_pool(name="qktp", bufs=2) as qktp,
            tc.tile_pool(name="vap", bufs=8) as vap,
            tc.tile_pool(name="att0p", bufs=4) as att0p,
            tc.tile_pool(name="attp", bufs=4) as attp,
            tc.tile_pool(name="aout", bufs=3) as aout,
            tc.tile_pool(name="rcp", bufs=16) as rcp,
            tc.tile_pool(name="atp", bufs=3) as atp,
            tc.tile_pool(name="outp", bufs=3) as outp,
            tc.tile_pool(name="acc", bufs=2, space="PSUM") as acc,
            tc.tile_pool(name="scps", bufs=2, space="PSUM") as scps,
            tc.tile_pool(name="aops", bufs=2, space="PSUM") as aops,
            tc.tile_pool(name="tp", bufs=2, space="PSUM") as tp,
        ):
            # ---- resident constants, DMA'd in first-use order -----------
            wts = [const.tile([128, 3 * D], F16, name=f"wk{k}") for k in range(8)]
            # q columns first (the first matmuls of the kernel), then k,
            # then v, in [128, 1024] pieces (2KB DMA lines) so the first q
            # matmul starts after ~2 MB instead of 8.4 MB.
            for which in range(3):
                c0 = which * D
                for k in range(8):
                    nc.sync.dma_start(
                        out=wts[k][:, c0:c0 + D],
                        in_=w_ext[k * 128:(k + 1) * 128, c0:c0 + D])
            e2t = const.tile([128, H * 128], F16, name="e2t")
            nc.sync.dma_start(out=e2t[:], in_=e2_ext[:])
            ident = const.tile([128, 128], F16, name="ident")
            make_identity(nc, ident)
            pwts = []
            for k in range(8):
                pk = const.tile([128, D], F16, name=f"pk{k}")
                nc.sync.dma_start(out=pk[:], in_=pw_ext[k * 128:(k + 1) * 128, :])
                pwts.append(pk)
            if with_qkbias:
                qkb = const.tile([128, 16], F32, name="qkb")
                for m in range(16):
                    nc.sync.dma_start(out=qkb[:, m:m + 1], in_=qkb_ext[m])
            if with_projbias:
                pbb = const.tile([128, D], F32, name="pbb")
                nc.sync.dma_start(out=pbb[:], in_=pbb_ext[:])

            # ---- main loop over s-tiles ---------------------------------
            for st in range(n_stiles):
                s0 = st * STILE

                # stage 1: load xT (feature-major) straight from DRAM
                xt = xtp.tile([128, 8 * STILE], F16, name="xt")
                for k in range(8):
                    # second HWDGE ring (ACT) so x prefetch doesn't queue
                    # behind the 8.4 MB of weight DMAs on the sync ring
                    nc.scalar.dma_start(
                        out=xt[:, k * STILE:(k + 1) * STILE],
                        in_=xT_ext[k * 128:(k + 1) * 128, s0:s0 + STILE])

                # stage 2a: qT, kT (feature-major, fp16)
                qt = qktp.tile([128, 8 * STILE], F16, name="qt", tag="qt")
                kt = qktp.tile([128, 8 * STILE], F16, name="kt", tag="kt")
                for which, dst in ((0, qt), (1, kt)):
                    for m in range(8):
                        ac = acc.tile([128, STILE], F32, name="ac")
                        col0 = which * D + m * 128
                        for k in range(8):
                            nc.tensor.matmul(
                                ac[:],
                                wts[k][:, col0:col0 + 128],
                                xt[:, k * STILE:(k + 1) * STILE],
                                start=(k == 0), stop=(k == 7))
                        dsl = dst[:, m * STILE:(m + 1) * STILE]
                        if with_qkbias:
                            nc.scalar.activation(
                                dsl, ac[:], mybir.ActivationFunctionType.Identity,
                                bias=qkb[:, which * 8 + m:which * 8 + m + 1])
                        else:
                            # evict on DVE: the ACT queue carries exp work and
                            # would stall the next PE accumulation group
                            nc.vector.tensor_copy(dsl, ac[:])

                # stage 2b: v (seq-major, ones column appended per head)
                vts = []
                for b in range(NBLK):
                    vt = vap.tile([128, H * (HD + 1)], F16, name="vt")
                    vt3 = vt.rearrange("p (h c) -> p h c", c=HD + 1)
                    nc.vector.memset(vt3[:, :, HD:HD + 1], 1.0)
                    for n in range(2):
                        ac = acc.tile([128, STILE], F32, name="ac")
                        for k in range(8):
                            nc.tensor.matmul(
                                ac[:],
                                xt[:, k * STILE + b * 128:k * STILE + b * 128 + 128],
                                wts[k][:, 2 * D + n * 512:2 * D + (n + 1) * 512],
                                start=(k == 0), stop=(k == 7))
                        nc.vector.tensor_copy(
                            vt3[:, n * 8:(n + 1) * 8, 0:HD],
                            ac.rearrange("p (h c) -> p h c", c=HD))
                    vts.append(vt)

                # stage 3: attention per window-pair block, 4 heads at a time
                for p in range(NBLK):
                    ao = aout.tile([128, D], F16, name="ao")
                    vt3 = vts[p].rearrange("p (h c) -> p h c", c=HD + 1)
                    for hg in range(4):
                        att0 = att0p.tile([128, 512], F16, name="att0")
                        att = attp.tile([128, 512], F16, name="att")
                        for i in range(4):
                            h = hg * 4 + i
                            m, r = h // 2, (h % 2) * 64
                            c0 = m * STILE + p * 128
                            # one PSUM tile per matmul group: sharing a bank
                            # between independent PE write-groups and readers
                            # crashes the hardware
                            sc = scps.tile([128, 128], F32, name="sc")
                            nc.tensor.matmul(
                                sc[:],
                                kt[r:r + 64, c0:c0 + 128],
                                qt[r:r + 64, c0:c0 + 128],
                                start=True, stop=True)
                            nc.scalar.activation(
                                att0[:, i * 128:(i + 1) * 128], sc[:],
                                mybir.ActivationFunctionType.Exp)
                        nc.vector.tensor_mul(
                            att[:], att0[:],
                            e2t[:, hg * 512:(hg + 1) * 512])
                        for i in range(4):
                            h = hg * 4 + i
                            aop = aops.tile([128, 128], F32, name="aop")
                            nc.tensor.matmul(
                                aop[:, :65],
                                att[:, i * 128:(i + 1) * 128],
                                vt3[:, h, :],
                                start=True, stop=True)
                            rc = rcp.tile([128, 1], F32, name="rc")
                            nc.vector.reciprocal(rc[:], aop[:, 64:65])
                            nc.scalar.activation(
                                ao[:, h * 64:(h + 1) * 64],
                                aop[:, :64],
                                mybir.ActivationFunctionType.Copy,
                                scale=rc[:])

                    # stage 4: transpose attn_out (PE), evict copies split
                    # DVE/ACT to balance load, proj matmul, store
                    at = atp.tile([128, D], F16, name="at")
                    for c in range(8):
                        tpp = tp.tile([128, 128], F16, name="tpp")
                        nc.tensor.transpose(
                            tpp[:], ao[:, c * 128:(c + 1) * 128], ident[:])
                        nc.scalar.copy(
                            at[:, c * 128:(c + 1) * 128], tpp[:])
                    ot = outp.tile([128, D], F32, name="ot")
                    for n in range(2):
                        ac = acc.tile([128, STILE], F32, name="ac")
                        for k in range(8):
                            nc.tensor.matmul(
                                ac[:],
                                at[:, k * 128:(k + 1) * 128],
                                pwts[k][:, n * 512:(n + 1) * 512],
                                start=(k == 0), stop=(k == 7))
                        nc.scalar.copy(ot[:, n * 512:(n + 1) * 512], ac[:])
                        if not with_projbias:
                            nc.sync.dma_start(
                                out=out_ext[s0 + p * 128:s0 + (p + 1) * 128,
                                            n * 512:(n + 1) * 512],
                                in_=ot[:, n * 512:(n + 1) * 512])
                    if with_projbias:
                        nc.vector.tensor_add(ot[:], ot[:], pbb[:])
                        nc.sync.dma_start(
                            out=out_ext[s0 + p * 128:s0 + (p + 1) * 128, :],
                            in_=ot[:])

    nc.compile()
    return nc


def _host_prep(x, qkv_w, qkv_b, proj_w, proj_b, rel_bias):
    """Fold scale/biases, cast to fp16, build the blocked exp-bias table."""
    scale = 1.0 / np.sqrt(HD)
    qkv_w_s = np.asarray(qkv_w, dtype=np.float64).copy()
    qkv_w_s[:, :D] *= scale
    qkv_b = np.asarray(qkv_b, dtype=np.float64)
    qkv_b_s = qkv_b.copy()
    qkv_b_s[:D] *= scale

    # rel-bias expanded to [H, W, W], exponentiated, then packed into the
    # transposed, window-pair-masked [128 (k), H*128 (h-major, q)] table.
    # exp(MASK) == 0 exactly, so the cross-window quadrants zero the
    # attention weights.
    rb = np.asarray(rel_bias, dtype=np.float32)
    coords = np.arange(W)
    rel = coords[:, None] - coords[None, :] + (W - 1)      # [q, k]
    bias_hqk = rb[rel].transpose(2, 0, 1)                  # [H, q, k]
    b2 = np.full((H, 128, 128), MASK, dtype=np.float32)    # [H, k2, q2]
    bias_kq = bias_hqk.transpose(0, 2, 1)                  # [H, k, q]
    b2[:, :64, :64] = bias_kq
    b2[:, 64:, 64:] = bias_kq
    e2 = np.exp(b2)
    expb2t16 = np.ascontiguousarray(
        e2.transpose(1, 0, 2).reshape(128, H * 128)).astype(NPDT)

    # v-bias commutes through attention (rows sum to 1) -> fold into proj_b
    proj_b_eff = (qkv_b[2 * D:] @ np.asarray(proj_w, dtype=np.float64)
                  + np.asarray(proj_b, dtype=np.float64))

    shared = {
        "qkvw16": qkv_w_s.astype(NPDT),
        "projw16": np.asarray(proj_w).astype(NPDT),
        "expb2t16": expb2t16,
    }
    qk_bias = qkv_b_s[:2 * D]
    with_qkbias = bool(np.any(qk_bias))
    if with_qkbias:
        shared["qkb"] = np.ascontiguousarray(
            qk_bias.reshape(16, 128, 1).astype(np.float32))
    with_projbias = bool(np.any(proj_b_eff))
    if with_projbias:
        shared["projb_bcast"] = np.broadcast_to(
            proj_b_eff.astype(np.float32), (128, D)).copy()
    return shared, with_qkbias, with_projbias


_NC_CACHE = {}


def kernel(x, qkv_w, qkv_b, proj_w, proj_b, rel_bias):
    x = np.asarray(x)
    shared, wqk, wpb = _host_prep(x, qkv_w, qkv_b, proj_w, proj_b, rel_bias)

    key = (wqk, wpb)
    if key not in _NC_CACHE:
        _NC_CACHE[key] = _build(NST, wqk, wpb)
    nc = _NC_CACHE[key]

    xT16 = np.ascontiguousarray(
        x.astype(NPDT).transpose(0, 2, 1))          # [B, D, S]
    in_maps = [dict(shared, xt16=xT16[i]) for i in range(B)]
    res = run_bass_kernel_spmd(nc, in_maps, list(range(B)))
    return np.stack([res.results[i]["out"] for i in range(B)], axis=0)


if __name__ == "__main__":
    rng = np.random.default_rng(0)
    x = rng.standard_normal((B, S, D), dtype=np.float32)
    qkv_w = rng.standard_normal((D, 3 * D), dtype=np.float32) / np.sqrt(D)
    proj_w = rng.standard_normal((D, D), dtype=np.float32) / np.sqrt(D)
    out = kernel(x, qkv_w, np.zeros(3 * D, np.float32), proj_w,
                 np.zeros(D, np.float32),
                 rng.standard_normal((2 * W - 1, H), dtype=np.float32) * 0.02)
    print(out.shape, out.dtype)


# revision 14
# speedup vs baseline: 1.4753x; 1.4753x over previous
"""LocalWindowAttention Trainium2 kernel (v2).

Problem: B=8, S=4096, D=1024, H=16 heads, hd=64, window W=64.
  qkv = x @ qkv_w + qkv_b; per-window attention with relative position
  bias; out = attn_out @ proj_w + proj_b.

Sharding: data-parallel over batch — one batch element per NeuronCore
(8 cores), no collectives needed.

v2 changes vs v1 (all aimed at keeping the PE instruction stream dense so
the tensor engine stays at its top p-state, and at cutting DVE/ACT work):
  1. x is uploaded pre-transposed (feature-major) from the host — the 256
     PE transposes + 256 DVE copies of stage 1 are gone.
  2. rel-pos bias is applied multiplicatively AFTER exp:
     exp(s+b) = exp(s)*exp(b). ACT reads score PSUM tiles directly
     (no DVE fp32 bias-add, no scb SBUF round-trip); the exp(b) table
     multiply is one fp16 all-SBUF DVE op per head-group (4x DVE mode).
     Masked cross-window quadrants have exp(-1e4)=0 exactly.
  3. softmax normalize is a single DVE tensor_scalar divide per head
     (out = unnorm/denom), replacing reciprocal+tensor_scalar_mul.
  4. attn-out transpose for the proj matmul runs on the DMA XBAR
     (SBUF->SBUF dma_start_transpose) instead of PE+DVE.
  5. qkv weights are DMA'd in first-use order (q column-chunks first) so
     the first q matmul starts ~1 us in instead of waiting for 6 MB.
  6. PSUM banks freed by (1)/(4) deepen the score/attn-v pools (3 deep).

Per-core pipeline (S=4096 rows, processed in s-tiles of 512 rows):
  qT/kT (feature-major) and v (seq-major) via fp16 matmuls vs resident
  qkv_w tiles; fp32 PSUM accumulation over the K=1024 contraction.
  Attention per 128-row block (= 2 windows of 64, independent via the
  zeroed quadrants of the exp-bias table) and per group of 4 heads:
      scoresT[k,q] = kT.T @ qT          (PE, PSUM)
      att0 = exp(scoresT)               (ACT, PSUM->SBUF fp16)
      att = att0 * exp_bias             (DVE fp16, 4x mode)
      outT_unnorm[q,hd], denom[q] = attT.T @ [v | 1]   (ones column fused)
      attn_out[q,hd] = outT_unnorm / denom             (DVE divide)
  attn_out DMA-transposed per 128-col block; proj matmul; DMA out.

Scale 1/sqrt(hd) is folded into qkv_w's q-columns host-side. qkv_b's
v-part is folded into an effective proj bias host-side (rows of attn sum
to 1). All matmul operands are fp16 (error ~8e-4 vs fp32 reference);
accumulation is always fp32.
"""
import numpy as np

import concourse.bacc as bacc
import concourse.mybir as mybir
from concourse.tile import TileContext
from concourse.bass_utils import run_bass_kernel_spmd
from concourse.masks import make_identity

import os
# PE operand dtype: bf16 by default (smaller mantissa multiplier draws less
# PE power than fp16 -> less power-throttling at the chip cap). Error is
# ~6e-3 vs the 2e-2 gate. Set BASS_LWA_FP16=1 to switch back to fp16.
if os.environ.get("BASS_LWA_FP16"):
    F16 = mybir.dt.float16
    NPDT = np.float16
else:
    F16 = mybir.dt.bfloat16
    NPDT = None  # ml_dtypes.bfloat16, resolved lazily below
    import ml_dtypes
    NPDT = ml_dtypes.bfloat16
F32 = mybir.dt.float32

B, S, D = 8, 4096, 1024
H, W, HD = 16, 64, 64
NW = S // W              # 64 windows
STILE = 512              # seq rows per pipeline tile
NST = S // STILE         # 8 s-tiles
NBLK = STILE // 128      # 4 row-blocks (window pairs) per s-tile
MASK = -10000.0          # exp() underflows to exactly 0 in fp16/fp32


def _build(n_stiles=NST, with_qkbias=False, with_projbias=False):
    nc = bacc.Bacc()
    s_total = n_stiles * STILE

    xT_ext = nc.declare_dram_parameter("xt16", [D, s_total], F16, isOutput=False)
    w_ext = nc.declare_dram_parameter("qkvw16", [D, 3 * D], F16, isOutput=False)
    pw_ext = nc.declare_dram_parameter("projw16", [D, D], F16, isOutput=False)
    e2_ext = nc.declare_dram_parameter("expb2t16", [128, H * 128], F16,
                                       isOutput=False)
    out_ext = nc.declare_dram_parameter("out", [s_total, D], F32, isOutput=True)
    if with_qkbias:
        qkb_ext = nc.declare_dram_parameter("qkb", [16, 128, 1], F32,
                                            isOutput=False)
    if with_projbias:
        pbb_ext = nc.declare_dram_parameter("projb_bcast", [128, D], F32,
                                            isOutput=False)

    with TileContext(nc) as tc:
        with (
            tc.tile_pool(name="const", bufs=1) as const,
            tc.tile_pool(name="xtp", bufs=3) as xtp,
            tc.tile_pool(name="qktp", bufs=2) as qktp,
            tc.tile_pool(name="vap", bufs=8) as vap,
            tc.tile_pool(name="att0p", bufs=4) as att0p,
            tc.tile_pool(name="attp", bufs=4) as attp,
            tc.tile_pool(name="aout", bufs=3) as aout,
            tc.tile_pool(name="rcp", bufs=16) as rcp,
            tc.tile_pool(name="atp", bufs=3) as atp,
            tc.tile_pool(name="outp", bufs=3) as outp,
            tc.tile_pool(name="acc", bufs=2, space="PSUM") as acc,
            tc.tile_pool(name="scps", bufs=1, space="PSUM") as scps,
            tc.tile_pool(name="aops", bufs=2, space="PSUM") as aops,
            tc.tile_pool(name="tp", bufs=2, space="PSUM") as tp,
        ):
            # ---- resident constants, DMA'd in first-use order -----------
            wts = [const.tile([128, 3 * D], F16, name=f"wk{k}") for k in range(8)]
            # q columns first (the first matmuls of the kernel), then k,
            # then v, in [128, 1024] pieces (2KB DMA lines) so the first q
            # matmul starts after ~2 MB instead of 8.4 MB.
            for which in range(3):
                c0 = which * D
                for k in range(8):
                    nc.sync.dma_start(
                        out=wts[k][:, c0:c0 + D],
                        in_=w_ext[k * 128:(k + 1) * 128, c0:c0 + D])
            e2t = const.tile([128, H * 128], F16, name="e2t")
            nc.sync.dma_start(out=e2t[:], in_=e2_ext[:])
            ident = const.tile([128, 128], F16, name="ident")
            make_identity(nc, ident)
            pwts = []
            for k in range(8):
                pk = const.tile([128, D], F16, name=f"pk{k}")
                nc.sync.dma_start(out=pk[:], in_=pw_ext[k * 128:(k + 1) * 128, :])
                pwts.append(pk)
            if with_qkbias:
                qkb = const.tile([128, 16], F32, name="qkb")
                for m in range(16):
                    nc.sync.dma_start(out=qkb[:, m:m + 1], in_=qkb_ext[m])
            if with_projbias:
                pbb = const.tile([128, D], F32, name="pbb")
                nc.sync.dma_start(out=pbb[:], in_=pbb_ext[:])

            # ---- software-pipelined main loop ---------------------------
            # The Tile scheduler freezes per-engine instruction order at
            # compile time, so the emission order must already interleave
            # the next s-tile's dense QKV groups into the attention phase:
            # otherwise the PE idles between attention dependencies, the
            # HAM activity monitor sees an idle window, and the PE clock
            # drops to 1.2 GHz for the whole phase.
            xts, qts, kts, vts = {}, {}, {}, {}

            def emit_xt(st):
                xt = xtp.tile([128, 8 * STILE], F16, name="xt")
                for k in range(8):
                    # second HWDGE ring (ACT) so x prefetch doesn't queue
                    # behind the 8.4 MB of weight DMAs on the sync ring
                    nc.scalar.dma_start(
                        out=xt[:, k * STILE:(k + 1) * STILE],
                        in_=xT_ext[k * 128:(k + 1) * 128,
                                   st * STILE:(st + 1) * STILE])
                xts[st] = xt

            def emit_qkv_group(st, g):
                """One 8-step accumulation group: g in [0,16) -> q/k m-tile,
                g in [16,24) -> v (block, half)."""
                xt = xts[st]
                if g == 0:
                    qts[st] = qktp.tile([128, 8 * STILE], F16, name="qt",
                                        tag="qt")
                    kts[st] = qktp.tile([128, 8 * STILE], F16, name="kt",
                                        tag="kt")
                    vts[st] = []
                if g < 16:
                    which, m = divmod(g, 8)
                    dst = (qts[st], kts[st])[which]
                    ac = acc.tile([128, STILE], F32, name="ac")
                    col0 = which * D + m * 128
                    for k in range(8):
                        nc.tensor.matmul(
                            ac[:],
                            wts[k][:, col0:col0 + 128],
                            xt[:, k * STILE:(k + 1) * STILE],
                            start=(k == 0), stop=(k == 7))
                    dsl = dst[:, m * STILE:(m + 1) * STILE]
                    if with_qkbias:
                        nc.scalar.activation(
                            dsl, ac[:], mybir.ActivationFunctionType.Identity,
                            bias=qkb[:, which * 8 + m:which * 8 + m + 1])
                    else:
                        nc.scalar.copy(dsl, ac[:])
                else:
                    b, n = divmod(g - 16, 2)
                    if n == 0:
                        vt = vap.tile([128, H * (HD + 1)], F16, name="vt")
                        vt3 = vt.rearrange("p (h c) -> p h c", c=HD + 1)
                        nc.vector.memset(vt3[:, :, HD:HD + 1], 1.0)
                        vts[st].append(vt)
                    vt3 = vts[st][b].rearrange("p (h c) -> p h c", c=HD + 1)
                    ac = acc.tile([128, STILE], F32, name="ac")
                    for k in range(8):
                        nc.tensor.matmul(
                            ac[:],
                            xt[:, k * STILE + b * 128:k * STILE + b * 128 + 128],
                            wts[k][:, 2 * D + n * 512:2 * D + (n + 1) * 512],
                            start=(k == 0), stop=(k == 7))
                    nc.vector.tensor_copy(
                        vt3[:, n * 8:(n + 1) * 8, 0:HD],
                        ac.rearrange("p (h c) -> p h c", c=HD))

            # within each head-group the two row-tile-concurrent score
            # matmul pairs land even heads in bank A, odd heads in bank B;
            # PERM is the resulting head processing order (e2t is permuted
            # to match host-side)
            PERM = (0, 2, 1, 3)

            def emit_attention_block(st, p, feed):
                s0 = st * STILE
                qt, kt = qts[st], kts[st]
                vt3 = vts[st][p].rearrange("p (h c) -> p h c", c=HD + 1)
                ao = aout.tile([128, D], F16, name="ao")
                for hg in range(4):
                    # scores: 2 concurrent row-tiles (K=64 halves of the PE
                    # array, auto tile_position from base_partition) x 2
                    # m-tiles; each bank collects 2 heads so one exp covers
                    # [128, 256]
                    scA = scps.tile([128, 256], F32, name="scA")
                    scB = scps.tile([128, 256], F32, name="scB")
                    for j in range(2):
                        m = 2 * hg + j
                        c0 = m * STILE + p * 128
                        nc.tensor.matmul(
                            scA[:, j * 128:(j + 1) * 128],
                            kt[0:64, c0:c0 + 128],
                            qt[0:64, c0:c0 + 128],
                            start=True, stop=True)
                        nc.tensor.matmul(
                            scB[:, j * 128:(j + 1) * 128],
                            kt[64:128, c0:c0 + 128],
                            qt[64:128, c0:c0 + 128],
                            start=True, stop=True)
                    att0 = att0p.tile([128, 512], F16, name="att0")
                    att = attp.tile([128, 512], F16, name="att")
                    nc.scalar.activation(
                        att0[:, 0:256], scA[:],
                        mybir.ActivationFunctionType.Exp)
                    nc.scalar.activation(
                        att0[:, 256:512], scB[:],
                        mybir.ActivationFunctionType.Exp)
                    nc.vector.tensor_mul(
                        att[:], att0[:], e2t[:, hg * 512:(hg + 1) * 512])
                    if feed:
                        emit_qkv_group(*feed.pop(0))
                    for i in range(4):
                        h = hg * 4 + PERM[i]
                        aop = aops.tile([128, 128], F32, name="aop")
                        nc.tensor.matmul(
                            aop[:, :65],
                            att[:, i * 128:(i + 1) * 128],
                            vt3[:, h, :],
                            start=True, stop=True)
                        rc = rcp.tile([128, 1], F32, name="rc")
                        nc.vector.reciprocal(rc[:], aop[:, 64:65])
                        if i % 2 == 0:
                            nc.scalar.activation(
                                ao[:, h * 64:(h + 1) * 64],
                                aop[:, :64],
                                mybir.ActivationFunctionType.Copy,
                                scale=rc[:])
                        else:
                            nc.vector.tensor_scalar_mul(
                                ao[:, h * 64:(h + 1) * 64],
                                aop[:, :64],
                                rc[:])
                for _ in range(2):
                    if feed:
                        emit_qkv_group(*feed.pop(0))
                at = atp.tile([128, D], F16, name="at")
                for c in range(8):
                    tpp = tp.tile([128, 128], F16, name="tpp")
                    nc.tensor.transpose(
                        tpp[:], ao[:, c * 128:(c + 1) * 128], ident[:])
                    if c % 2 == 0:
                        nc.vector.tensor_copy(
                            at[:, c * 128:(c + 1) * 128], tpp[:])
                    else:
                        nc.scalar.copy(
                            at[:, c * 128:(c + 1) * 128], tpp[:])
                ot = outp.tile([128, D], F32, name="ot")
                for n in range(2):
                    ac = acc.tile([128, STILE], F32, name="ac")
                    for k in range(8):
                        nc.tensor.matmul(
                            ac[:],
                            at[:, k * 128:(k + 1) * 128],
                            pwts[k][:, n * 512:(n + 1) * 512],
                            start=(k == 0), stop=(k == 7))
                    nc.scalar.copy(ot[:, n * 512:(n + 1) * 512], ac[:])
                    if not with_projbias:
                        nc.sync.dma_start(
                            out=out_ext[s0 + p * 128:s0 + (p + 1) * 128,
                                        n * 512:(n + 1) * 512],
                            in_=ot[:, n * 512:(n + 1) * 512])
                if with_projbias:
                    nc.vector.tensor_add(ot[:], ot[:], pbb[:])
                    nc.sync.dma_start(
                        out=out_ext[s0 + p * 128:s0 + (p + 1) * 128, :],
                        in_=ot[:])

            emit_xt(0)
            for g in range(24):
                emit_qkv_group(0, g)
            for st in range(n_stiles):
                if st + 1 < n_stiles:
                    emit_xt(st + 1)
                    feed = [(st + 1, g) for g in range(24)]
                else:
                    feed = []
                for p in range(NBLK):
                    emit_attention_block(st, p, feed)
                while feed:
                    emit_qkv_group(*feed.pop(0))

    nc.compile()
    return nc


def _host_prep(x, qkv_w, qkv_b, proj_w, proj_b, rel_bias):
    """Fold scale/biases, cast to fp16, build the blocked exp-bias table."""
    scale = 1.0 / np.sqrt(HD)
    qkv_w_s = np.asarray(qkv_w, dtype=np.float64).copy()
    qkv_w_s[:, :D] *= scale
    qkv_b = np.asarray(qkv_b, dtype=np.float64)
    qkv_b_s = qkv_b.copy()
    qkv_b_s[:D] *= scale

    # rel-bias expanded to [H, W, W], exponentiated, then packed into the
    # transposed, window-pair-masked [128 (k), H*128 (h-major, q)] table.
    # exp(MASK) == 0 exactly, so the cross-window quadrants zero the
    # attention weights.
    rb = np.asarray(rel_bias, dtype=np.float32)
    coords = np.arange(W)
    rel = coords[:, None] - coords[None, :] + (W - 1)      # [q, k]
    bias_hqk = rb[rel].transpose(2, 0, 1)                  # [H, q, k]
    b2 = np.full((H, 128, 128), MASK, dtype=np.float32)    # [H, k2, q2]
    bias_kq = bias_hqk.transpose(0, 2, 1)                  # [H, k, q]
    b2[:, :64, :64] = bias_kq
    b2[:, 64:, 64:] = bias_kq
    e2 = np.exp(b2)
    # head order within each group of 4 matches the kernel's bank layout
    # (even heads in bank A, odd in bank B -> processing order 0,2,1,3)
    head_order = np.concatenate([4 * g + np.array([0, 2, 1, 3])
                                 for g in range(4)])
    e2 = e2[head_order]
    expb2t16 = np.ascontiguousarray(
        e2.transpose(1, 0, 2).reshape(128, H * 128)).astype(NPDT)

    # v-bias commutes through attention (rows sum to 1) -> fold into proj_b
    proj_b_eff = (qkv_b[2 * D:] @ np.asarray(proj_w, dtype=np.float64)
                  + np.asarray(proj_b, dtype=np.float64))

    shared = {
        "qkvw16": qkv_w_s.astype(NPDT),
        "projw16": np.asarray(proj_w).astype(NPDT),
        "expb2t16": expb2t16,
    }
    qk_bias = qkv_b_s[:2 * D]
    with_qkbias = bool(np.any(qk_bias))
    if with_qkbias:
        shared["qkb"] = np.ascontiguousarray(
            qk_bias.reshape(16, 128, 1).astype(np.float32))
    with_projbias = bool(np.any(proj_b_eff))
    if with_projbias:
        shared["projb_bcast"] = np.broadcast_to(
            proj_b_eff.astype(np.float32), (128, D)).copy()
    return shared, with_qkbias, with_projbias


_NC_CACHE = {}


def kernel(x, qkv_w, qkv_b, proj_w, proj_b, rel_bias):
    x = np.asarray(x)
    shared, wqk, wpb = _host_prep(x, qkv_w, qkv_b, proj_w, proj_b, rel_bias)

    key = (wqk, wpb)
    if key not in _NC_CACHE:
        _NC_CACHE[key] = _build(NST, wqk, wpb)
    nc = _NC_CACHE[key]

    xT16 = np.ascontiguousarray(
        x.astype(NPDT).transpose(0, 2, 1))          # [B, D, S]
    in_maps = [dict(shared, xt16=xT16[i]) for i in range(B)]
    res = run_bass_kernel_spmd(nc, in_maps, list(range(B)))
    return np.stack([res.results[i]["out"] for i in range(B)], axis=0)


if __name__ == "__main__":
    rng = np.random.default_rng(0)
    x = rng.standard_normal((B, S, D), dtype=np.float32)
    qkv_w = rng.standard_normal((D, 3 * D), dtype=np.float32) / np.sqrt(D)
    proj_w = rng.standard_normal((D, D), dtype=np.float32) / np.sqrt(D)
    out = kernel(x, qkv_w, np.zeros(3 * D, np.float32), proj_w,
                 np.zeros(D, np.float32),
                 rng.standard_normal((2 * W - 1, H), dtype=np.float32) * 0.02)
    print(out.shape, out.dtype)
